# revision 12
# baseline (speedup 1.0000x reference)
"""Trainium2 Bass kernel for nn_DMSK_26285199851632 (moe_routing).

Strategy: pure data-parallel over batch (16 samples / 8 cores = 2 per core).
Per core, the 2 samples' 64 post-projection channels are stacked on the 128
SBUF partitions (p = 64*s + c).  The argmax routing is computed on-device
with mask-merged expert weights (smaller kernels zero-padded into the
largest, center-aligned — valid because route-2 experts share dilation).
Depthwise conv taps are split between the TensorEngine (diagonal-matmul
accumulation in PSUM) and the VectorEngine (fused scalar_tensor_tensor
MAC chains in bf16), which run concurrently.
"""

import numpy as np
import ml_dtypes

bf16 = ml_dtypes.bfloat16

B, C, H, W = 16, 128, 160, 160
C2 = C // 2
NCORES = 8
BPC = B // NCORES          # samples per core
HW = H * W
PAD1 = 2                   # route-1 merged kernel 5x5, dilation 1
H1P, W1P = H + 2 * PAD1, W + 2 * PAD1
PAD2 = 15                  # route-2 merged kernel 11x11, dilation 3
H2P, W2P = H + 2 * PAD2, W + 2 * PAD2
PAD3 = 3                   # spatial 7x7 conv
H3P, W3P = H + 2 * PAD3, W + 2 * PAD3

# row tiles: 53 x 3 rows + 1 x 1 row
TILES = [(3 * i, 3) for i in range(53)] + [(159, 1)]


def _dve_tile(t):
    """Tiles whose conv taps run on the VectorEngine (rest on TensorEngine)."""
    return t % 3 == 0


_PROGRAM_CACHE = {}


def _build_program():
    import concourse.bacc as bacc
    import concourse.tile as tile
    import concourse.bass as bass
    import concourse.bass_isa as bass_isa
    from concourse import mybir
    from contextlib import ExitStack

    f32 = mybir.dt.float32
    bf = mybir.dt.bfloat16
    Alu = mybir.AluOpType
    Act = mybir.ActivationFunctionType
    Ax = mybir.AxisListType

    nc = bacc.Bacc("TRN2", target_bir_lowering=False, debug=False,
                   num_devices=NCORES)

    xd = nc.declare_dram_parameter("x", [BPC, C, H, W], f32, isOutput=False)
    projT_d = nc.declare_dram_parameter("projT", [C, C2], bf, isOutput=False)
    w1p_d = nc.declare_dram_parameter("w1p", [128, 2, 25], bf, isOutput=False)
    w2p_d = nc.declare_dram_parameter("w2p", [128, 3, 121], bf, isOutput=False)
    A1_d = nc.declare_dram_parameter("A1", [128, 2, 128], f32, isOutput=False)
    A2_d = nc.declare_dram_parameter("A2", [128, 3, 128], f32, isOutput=False)
    b1_d = nc.declare_dram_parameter("b1", [128, 2], f32, isOutput=False)
    b2_d = nc.declare_dram_parameter("b2", [128, 3], f32, isOutput=False)
    eye_d = nc.declare_dram_parameter("eye", [128, 128], bf, isOutput=False)
    M1_d = nc.declare_dram_parameter("M1", [128, 2], bf, isOutput=False)
    G_d = nc.declare_dram_parameter("G", [2, 2, 128], bf, isOutput=False)
    spwT_d = nc.declare_dram_parameter("spwT", [98, 2], bf, isOutput=False)
    spb_d = nc.declare_dram_parameter("spb", [2, 1], f32, isOutput=False)
    outd = nc.declare_dram_parameter("out", [BPC, C, H, W], f32, isOutput=True)

    with tile.TileContext(nc) as tc, ExitStack() as ctx:
        g = ctx.enter_context(tc.tile_pool(name="glob", bufs=1))

        # ---- persistent buffers ----
        xp2 = g.tile([128, H2P, W2P], bf, name="xp2")       # att1, padded
        att2 = g.tile([128, H, W], bf, name="att2")
        pool1p = g.tile([128, 64], f32, name="pool1p")
        pool2p = g.tile([128, 64], f32, name="pool2p")

        # ---- aux constants into SBUF ----
        projT = g.tile([C, C2], bf, name="projT_sb")
        nc.sync.dma_start(out=projT[:], in_=projT_d[:])
        w1p = g.tile([128, 2, 25], bf, name="w1p_sb")
        nc.sync.dma_start(out=w1p[:], in_=w1p_d[:])
        w2p = g.tile([128, 3, 121], bf, name="w2p_sb")
        nc.sync.dma_start(out=w2p[:], in_=w2p_d[:])
        A1 = g.tile([128, 2, 128], f32, name="A1_sb")
        nc.sync.dma_start(out=A1[:], in_=A1_d[:])
        A2 = g.tile([128, 3, 128], f32, name="A2_sb")
        nc.sync.dma_start(out=A2[:], in_=A2_d[:])
        b1 = g.tile([128, 2], f32, name="b1_sb")
        nc.sync.dma_start(out=b1[:], in_=b1_d[:])
        b2 = g.tile([128, 3], f32, name="b2_sb")
        nc.sync.dma_start(out=b2[:], in_=b2_d[:])
        eye = g.tile([128, 128], bf, name="eye_sb")
        nc.sync.dma_start(out=eye[:], in_=eye_d[:])
        M1 = g.tile([128, 2], bf, name="M1_sb")
        nc.sync.dma_start(out=M1[:], in_=M1_d[:])
        G = g.tile([2, 2, 128], bf, name="G_sb")
        nc.sync.dma_start(out=G[:], in_=G_d[:])
        spwT = g.tile([98, 2], bf, name="spwT_sb")
        nc.sync.dma_start(out=spwT[:], in_=spwT_d[:])
        spb = g.tile([2, 1], f32, name="spb_sb")
        nc.sync.dma_start(out=spb[:], in_=spb_d[:])

        # routing scratch (tiny)
        pool1 = g.tile([128, 1], f32, name="pool1")
        pool2 = g.tile([128, 1], f32, name="pool2")
        lg = g.tile([128, 3], f32, name="lg")
        mx = g.tile([128, 1], f32, name="mx")
        msk = g.tile([128, 3], f32, name="msk")
        w1m_a = g.tile([128, 25], f32, name="w1m_a")
        w1m = g.tile([128, 25], f32, name="w1m")
        w2m_a = g.tile([128, 121], f32, name="w2m_a")
        w2m = g.tile([128, 121], f32, name="w2m")

        # zero the pad borders of xp2 (interior fully written by route 1)
        nc.gpsimd.memset(xp2[:, 0:PAD2, :], 0.0)
        nc.gpsimd.memset(xp2[:, H + PAD2:, :], 0.0)
        nc.gpsimd.memset(xp2[:, PAD2:PAD2 + H, 0:PAD2], 0.0)
        nc.gpsimd.memset(xp2[:, PAD2:PAD2 + H, W + PAD2:], 0.0)

        def conv_taps(t, r0, nr, src, wm, diag, taps, kw, dil, out_win, pool_col,
                      ps_pool, acc_pool):
            """Emit one row-tile of merged-kernel depthwise conv."""
            n_t = len(taps)
            if not _dve_tile(t):
                ps = ps_pool.tile([128, nr, W], f32, tag="cps")
                for i, tap in enumerate(taps):
                    dy, dx = divmod(tap, kw)
                    rhs = src[:, r0 + dil * dy: r0 + dil * dy + nr,
                              dil * dx: dil * dx + W]
                    nc.tensor.matmul(out=ps[:], lhsT=diag[:, tap, :], rhs=rhs,
                                     start=(i == 0), stop=(i == n_t - 1))
                if pool_col is not None:
                    nc.scalar.activation(out=out_win, in_=ps[:], func=Act.Copy,
                                         accum_out=pool_col)
                else:
                    nc.scalar.activation(out=out_win, in_=ps[:], func=Act.Copy)
            else:
                acc = [acc_pool.tile([128, nr, W], bf, tag="acca", name="acca"),
                       acc_pool.tile([128, nr, W], bf, tag="accb", name="accb")]

                def win(tap):
                    dy, dx = divmod(tap, kw)
                    return src[:, r0 + dil * dy: r0 + dil * dy + nr,
                               dil * dx: dil * dx + W]

                nc.vector.tensor_scalar_mul(out=acc[0][:], in0=win(taps[0]),
                                            scalar1=wm[:, taps[0]:taps[0] + 1])
                cur = 0
                for tap in taps[1:-1]:
                    nxt = 1 - cur
                    nc.vector.scalar_tensor_tensor(
                        out=acc[nxt][:], in0=win(tap),
                        scalar=wm[:, tap:tap + 1], in1=acc[cur][:],
                        op0=Alu.mult, op1=Alu.add)
                    cur = nxt
                tap = taps[-1]
                kw_extra = {}
                if pool_col is not None:
                    kw_extra["accum_out"] = pool_col
                nc.vector.scalar_tensor_tensor(
                    out=out_win, in0=win(tap), scalar=wm[:, tap:tap + 1],
                    in1=acc[cur][:], op0=Alu.mult, op1=Alu.add, **kw_extra)

        with tc.tile_pool(name="pAD", bufs=1) as pAD:
            xp1 = pAD.tile([128, H1P, W1P], bf, name="xp1")
            diag1 = pAD.tile([128, 25, 128], bf, name="diag1")
            nc.gpsimd.memset(xp1[:, 0:PAD1, :], 0.0)
            nc.gpsimd.memset(xp1[:, H + PAD1:, :], 0.0)
            nc.gpsimd.memset(xp1[:, PAD1:PAD1 + H, 0:PAD1], 0.0)
            nc.gpsimd.memset(xp1[:, PAD1:PAD1 + H, W + PAD1:], 0.0)

            # ---- phase A: load x, cast, 1x1 projection, pooled sums ----
            with tc.tile_pool(name="pa", bufs=3) as pa, \
                 tc.tile_pool(name="pa_ps", bufs=4, space="PSUM") as pa_ps:
                for t, (r0, nr) in enumerate(TILES):
                    x_t = pa.tile([128, BPC, nr, W], f32, tag="x_t")
                    nc.sync.dma_start(
                        out=x_t[:],
                        in_=xd[:, :, r0:r0 + nr, :].rearrange(
                            "s c r w -> c s r w"))
                    x_bf = pa.tile([128, BPC, nr, W], bf, tag="x_bf")
                    if t % 2 == 0:
                        nc.vector.tensor_copy(out=x_bf[:], in_=x_t[:])
                    else:
                        nc.scalar.copy(out=x_bf[:], in_=x_t[:])
                    ps = pa_ps.tile([128, nr, W], f32, tag="ps")
                    nc.tensor.matmul(out=ps[0:C2], lhsT=projT[:],
                                     rhs=x_bf[:, 0], start=True, stop=True)
                    nc.tensor.matmul(out=ps[C2:128], lhsT=projT[:],
                                     rhs=x_bf[:, 1], start=True, stop=True)
                    nc.scalar.activation(
                        out=xp1[:, PAD1 + r0:PAD1 + r0 + nr, PAD1:PAD1 + W],
                        in_=ps[:], func=Act.Copy,
                        accum_out=pool1p[:, t:t + 1])

            # ---- routing 1 ----
            nc.vector.tensor_reduce(out=pool1[:], in_=pool1p[:, 0:len(TILES)],
                                    axis=Ax.X, op=Alu.add)
            with tc.tile_pool(name="rt1_ps", bufs=2, space="PSUM") as rt_ps:
                for e in range(2):
                    lg_ps = rt_ps.tile([128, 1], f32, tag="lg")
                    nc.tensor.matmul(out=lg_ps[:], lhsT=A1[:, e, :],
                                     rhs=pool1[:], start=True, stop=True)
                    nc.vector.tensor_scalar_add(out=lg[:, e:e + 1],
                                                in0=lg_ps[:],
                                                scalar1=b1[:, e:e + 1])
            nc.vector.tensor_tensor(out=mx[:], in0=lg[:, 0:1], in1=lg[:, 1:2],
                                    op=Alu.max)
            for e in range(2):
                nc.vector.tensor_tensor(out=msk[:, e:e + 1], in0=lg[:, e:e + 1],
                                        in1=mx[:], op=Alu.is_ge)
            nc.vector.tensor_scalar_mul(out=w1m_a[:], in0=w1p[:, 0, :],
                                        scalar1=msk[:, 0:1])
            nc.vector.scalar_tensor_tensor(out=w1m[:], in0=w1p[:, 1, :],
                                           scalar=msk[:, 1:2], in1=w1m_a[:],
                                           op0=Alu.mult, op1=Alu.add)
            nc.vector.tensor_tensor(
                out=diag1[:],
                in0=eye[:].unsqueeze(1).broadcast_to([128, 25, 128]),
                in1=w1m[:].unsqueeze(2).broadcast_to([128, 25, 128]),
                op=Alu.mult)

            # ---- phase D: route-1 conv (5x5, dilation 1) ----
            with tc.tile_pool(name="pd_ps", bufs=4, space="PSUM") as pd_ps, \
                 tc.tile_pool(name="pd_acc", bufs=2) as pd_acc:
                for t, (r0, nr) in enumerate(TILES):
                    conv_taps(
                        t, r0, nr, xp1, w1m, diag1, list(range(25)), 5, 1,
                        xp2[:, PAD2 + r0:PAD2 + r0 + nr, PAD2:PAD2 + W],
                        pool2p[:, t:t + 1], pd_ps, pd_acc)

        # ---- routing 2 ----
        with tc.tile_pool(name="pE", bufs=1) as pE:
            diag2 = pE.tile([128, 121, 128], bf, name="diag2")
            nc.vector.tensor_reduce(out=pool2[:], in_=pool2p[:, 0:len(TILES)],
                                    axis=Ax.X, op=Alu.add)
            with tc.tile_pool(name="rt2_ps", bufs=2, space="PSUM") as rt_ps:
                for e in range(3):
                    lg_ps = rt_ps.tile([128, 1], f32, tag="lg")
                    nc.tensor.matmul(out=lg_ps[:], lhsT=A2[:, e, :],
                                     rhs=pool2[:], start=True, stop=True)
                    nc.vector.tensor_scalar_add(out=lg[:, e:e + 1],
                                                in0=lg_ps[:],
                                                scalar1=b2[:, e:e + 1])
            nc.vector.tensor_tensor(out=mx[:], in0=lg[:, 0:1], in1=lg[:, 1:2],
                                    op=Alu.max)
            nc.vector.tensor_tensor(out=mx[:], in0=mx[:], in1=lg[:, 2:3],
                                    op=Alu.max)
            for e in range(3):
                nc.vector.tensor_tensor(out=msk[:, e:e + 1], in0=lg[:, e:e + 1],
                                        in1=mx[:], op=Alu.is_ge)
            nc.vector.tensor_scalar_mul(out=w2m_a[:], in0=w2p[:, 0, :],
                                        scalar1=msk[:, 0:1])
            nc.vector.scalar_tensor_tensor(out=w2m[:], in0=w2p[:, 1, :],
                                           scalar=msk[:, 1:2], in1=w2m_a[:],
                                           op0=Alu.mult, op1=Alu.add)
            nc.vector.scalar_tensor_tensor(out=w2m_a[:], in0=w2p[:, 2, :],
                                           scalar=msk[:, 2:3], in1=w2m[:],
                                           op0=Alu.mult, op1=Alu.add)
            nc.vector.tensor_tensor(
                out=diag2[:],
                in0=eye[:].unsqueeze(1).broadcast_to([128, 121, 128]),
                in1=w2m_a[:].unsqueeze(2).broadcast_to([128, 121, 128]),
                op=Alu.mult)

            # ---- phase E: route-2 conv (11x11, dilation 3) ----
            with tc.tile_pool(name="pe_ps", bufs=4, space="PSUM") as pe_ps, \
                 tc.tile_pool(name="pe_acc", bufs=2) as pe_acc:
                for t, (r0, nr) in enumerate(TILES):
                    conv_taps(
                        t, r0, nr, xp2, w2m_a, diag2, list(range(121)), 11, 3,
                        att2[:, r0:r0 + nr, :], None, pe_ps, pe_acc)

        # ---- phase F: gating + blend + output ----
        import concourse.bass_isa as bisa
        maps = nc.dram_tensor("maps_scratch", [4, H3P, W3P], bf)
        with tc.tile_pool(name="pF", bufs=1) as pF:
            # zero the pad borders of the DRAM maps buffer
            zpad = pF.tile([4, 512], bf, name="zpad")
            nc.gpsimd.memset(zpad[:], 0.0)
            nc.sync.dma_start(out=maps[:, 0:PAD3, :],
                              in_=zpad[:, 0:PAD3 * W3P].rearrange(
                                  "p (r w) -> p r w", r=PAD3))
            nc.sync.dma_start(out=maps[:, H + PAD3:, :],
                              in_=zpad[:, 0:PAD3 * W3P].rearrange(
                                  "p (r w) -> p r w", r=PAD3))
            nc.sync.dma_start(out=maps[:, PAD3:PAD3 + H, 0:PAD3],
                              in_=zpad[:, 0:H * PAD3].rearrange(
                                  "p (r w) -> p r w", r=H))
            nc.sync.dma_start(out=maps[:, PAD3:PAD3 + H, W + PAD3:],
                              in_=zpad[:, 0:H * PAD3].rearrange(
                                  "p (r w) -> p r w", r=H))

            # F pass 1: per-pixel channel mean(sum) and max per sample
            with tc.tile_pool(name="pf1", bufs=3) as pf1, \
                 tc.tile_pool(name="pf1_ps", bufs=2, space="PSUM") as pf1_ps:
                for t, (r0, nr) in enumerate(TILES):
                    att1w = xp2[:, PAD2 + r0:PAD2 + r0 + nr, PAD2:PAD2 + W]
                    att2w = att2[:, r0:r0 + nr, :]
                    mm_ps = pf1_ps.tile([2, nr, W], f32, tag="mm")
                    nc.tensor.matmul(out=mm_ps[:], lhsT=M1[:], rhs=att1w,
                                     start=True, stop=False)
                    nc.tensor.matmul(out=mm_ps[:], lhsT=M1[:], rhs=att2w,
                                     start=False, stop=True)
                    mstage = pf1.tile([2, nr, W], bf, tag="mstage")
                    nc.scalar.activation(out=mstage[:], in_=mm_ps[:],
                                         func=Act.Copy)
                    nc.sync.dma_start(
                        out=maps[0:2, PAD3 + r0:PAD3 + r0 + nr, PAD3:PAD3 + W],
                        in_=mstage[:])
                    mboth = pf1.tile([128, nr, W], bf, tag="mboth")
                    nc.vector.tensor_tensor(out=mboth[:], in0=att1w,
                                            in1=att2w, op=Alu.max)
                    pmx = pf1.tile([128, nr, W], bf, tag="pmx")
                    nc.gpsimd.partition_all_reduce(pmx[0:C2], mboth[0:C2], C2,
                                                   bisa.ReduceOp.max)
                    nc.gpsimd.partition_all_reduce(pmx[C2:128], mboth[C2:128],
                                                   C2, bisa.ReduceOp.max)
                    nc.sync.dma_start(
                        out=maps[2:3, PAD3 + r0:PAD3 + r0 + nr, PAD3:PAD3 + W],
                        in_=pmx[0:1])
                    nc.sync.dma_start(
                        out=maps[3:4, PAD3 + r0:PAD3 + r0 + nr, PAD3:PAD3 + W],
                        in_=pmx[C2:C2 + 1])

            # build the full 7x7 im2col of the maps in DRAM: X2[s, m*49+dy*7+dx,
            # r*W + w] = maps[s + 2m, r + dy, w + dx]
            x2d = nc.dram_tensor("x2_scratch", [BPC, 98, H * W], bf)
            for s in range(BPC):
                for m in range(2):
                    for dy in range(7):
                        src = bass.AP(
                            tensor=maps,
                            offset=(s + 2 * m) * (H3P * W3P) + dy * W3P,
                            ap=[[1, 7], [W3P, H], [1, W]])
                        dst = bass.AP(
                            tensor=x2d,
                            offset=s * (98 * H * W) + (m * 49 + dy * 7) * H * W,
                            ap=[[H * W, 7], [W, H], [1, W]])
                        nc.sync.dma_start(out=dst, in_=src)

            # F pass 2: spatial conv, sigmoid gate, blend, write out
            with tc.tile_pool(name="pf2", bufs=3) as pf2, \
                 tc.tile_pool(name="pf2_ps", bufs=2, space="PSUM") as pf2_ps:
                for t, (r0, nr) in enumerate(TILES):
                    att1w = xp2[:, PAD2 + r0:PAD2 + r0 + nr, PAD2:PAD2 + W]
                    att2w = att2[:, r0:r0 + nr, :]
                    sig = []
                    for s in range(BPC):
                        x2s = pf2.tile([98, nr, W], bf, tag=f"x2_{s}",
                                       name=f"x2_{s}")
                        nc.sync.dma_start(
                            out=x2s[:],
                            in_=x2d[s, :, r0 * W:(r0 + nr) * W].rearrange(
                                "p (r w) -> p r w", r=nr))
                        sp_ps = pf2_ps.tile([2, nr, W], f32, tag=f"sp{s}")
                        nc.tensor.matmul(out=sp_ps[:], lhsT=spwT[:],
                                         rhs=x2s[:], start=True, stop=True)
                        sg = pf2.tile([2, nr, W], bf, tag=f"sig{s}")
                        nc.scalar.activation(out=sg[:], in_=sp_ps[:],
                                             func=Act.Sigmoid, bias=spb[:],
                                             scale=1.0)
                        sig.append(sg)
                    gate_ps = pf2_ps.tile([128, nr, W], f32, tag="gps")
                    nc.tensor.matmul(out=gate_ps[:], lhsT=G[:, 0, :],
                                     rhs=sig[0][:], start=True, stop=False)
                    nc.tensor.matmul(out=gate_ps[:], lhsT=G[:, 1, :],
                                     rhs=sig[1][:], start=False, stop=True)
                    gate = pf2.tile([128, nr, W], bf, tag="gate")
                    nc.scalar.activation(out=gate[:], in_=gate_ps[:],
                                         func=Act.Copy)
                    tmp1 = pf2.tile([128, nr, W], bf, tag="tmp1")
                    nc.vector.tensor_tensor(out=tmp1[:], in0=att1w, in1=gate[:],
                                            op=Alu.mult)
                    tmp2 = pf2.tile([128, nr, W], bf, tag="tmp2")
                    nc.vector.tensor_tensor(out=tmp2[:], in0=att2w, in1=gate[:],
                                            op=Alu.mult)
                    xr1 = pf2.tile([128, nr, W], f32, tag="xr1")
                    xr2 = pf2.tile([128, nr, W], f32, tag="xr2")
                    for s in range(BPC):
                        nc.sync.dma_start(out=xr1[C2 * s:C2 * (s + 1)],
                                          in_=xd[s, 0:C2, r0:r0 + nr, :])
                        nc.sync.dma_start(out=xr2[C2 * s:C2 * (s + 1)],
                                          in_=xd[s, C2:C, r0:r0 + nr, :])
                    o1 = pf2.tile([128, nr, W], f32, tag="o1")
                    nc.vector.tensor_tensor(out=o1[:], in0=tmp1[:], in1=xr1[:],
                                            op=Alu.add)
                    o2 = pf2.tile([128, nr, W], f32, tag="o2")
                    nc.gpsimd.tensor_tensor(out=o2[:], in0=tmp2[:], in1=xr2[:],
                                            op=Alu.add)
                    for s in range(BPC):
                        nc.sync.dma_start(out=outd[s, 0:C2, r0:r0 + nr, :],
                                          in_=o1[C2 * s:C2 * (s + 1)])
                        nc.sync.dma_start(out=outd[s, C2:C, r0:r0 + nr, :],
                                          in_=o2[C2 * s:C2 * (s + 1)])

    nc.finalize()
    return nc


def _host_aux(inputs):
    """Precompute the auxiliary (replicated) input tensors on the host."""
    f = lambda k: np.asarray(inputs[k], dtype=np.float32)
    proj_w = f("proj_w")
    att1_w, att1_b = f("att1_w"), f("att1_b")
    att2_w, att2_b = f("att2_w"), f("att2_b")
    sp_w, sp_b = f("sp_w"), f("sp_b")

    aux = {}
    aux["projT"] = proj_w.T.astype(bf16)                       # (128, 64)

    w1 = np.zeros((2, C2, 5, 5), np.float32)
    w1[0, :, 1:4, 1:4] = f("dw1_w3")[:, 0]
    w1[1] = f("dw1_w5")[:, 0]
    pidx = np.arange(128) % C2
    aux["w1p"] = w1[:, pidx].transpose(1, 0, 2, 3).reshape(128, 2, 25).astype(bf16)

    w2 = np.zeros((3, C2, 11, 11), np.float32)
    w2[0, :, 2:9, 2:9] = f("dw2_w7")[:, 0]
    w2[1, :, 1:10, 1:10] = f("dw2_w9")[:, 0]
    w2[2] = f("dw2_w11")[:, 0]
    aux["w2p"] = w2[:, pidx].transpose(1, 0, 2, 3).reshape(128, 3, 121).astype(bf16)

    blk = np.arange(128) // C2
    same = (blk[:, None] == blk[None, :]).astype(np.float32)   # (k, p)
    A1 = np.zeros((128, 2, 128), np.float32)
    A2 = np.zeros((128, 3, 128), np.float32)
    for e in range(2):
        A1[:, e, :] = same * (att1_w[e, pidx] / HW)[:, None]
    for e in range(3):
        A2[:, e, :] = same * (att2_w[e, pidx] / HW)[:, None]
    aux["A1"], aux["A2"] = A1, A2
    aux["b1"] = np.tile(att1_b[None, :], (128, 1)).astype(np.float32)
    aux["b2"] = np.tile(att2_b[None, :], (128, 1)).astype(np.float32)
    aux["eye"] = np.eye(128, dtype=np.float32).astype(bf16)
    aux["M1"] = (blk[:, None] == np.arange(2)[None, :]).astype(np.float32).astype(bf16)
    Gm = np.zeros((2, 2, 128), np.float32)
    for s in range(2):
        Gm[s, :, :] = (blk[None, :] == s)
    aux["G"] = Gm.astype(bf16)
    spwT = np.zeros((98, 2), np.float32)
    for o in range(2):
        for i in range(2):
            scale = (1.0 / 128.0) if i == 0 else 1.0
            spwT[i * 49:(i + 1) * 49, o] = sp_w[o, i].reshape(49) * scale
    aux["spwT"] = spwT.astype(bf16)
    aux["spb"] = sp_b.reshape(2, 1).astype(np.float32)
    return aux


def kernel(**inputs):
    from concourse.bass_utils import run_bass_kernel_spmd

    if "nc" not in _PROGRAM_CACHE:
        _PROGRAM_CACHE["nc"] = _build_program()
    nc = _PROGRAM_CACHE["nc"]

    x = np.asarray(inputs["x"], dtype=np.float32)
    aux = _host_aux(inputs)
    in_maps = []
    for core in range(NCORES):
        m = {"x": np.ascontiguousarray(x[BPC * core: BPC * (core + 1)])}
        m.update(aux)
        in_maps.append(m)

    res = run_bass_kernel_spmd(nc, in_maps, list(range(NCORES)))
    out = np.empty((B, C, H, W), np.float32)
    for core in range(NCORES):
        out[BPC * core: BPC * (core + 1)] = res.results[core]["out"]
    return out
